# revision 38
# baseline (speedup 1.0000x reference)
"""KNN entropy loss (k=5, B=8192, D=768) on 8 TRN2 NeuronCores.

Each core owns 1024 rows of x and computes its [1024 x 8192] block of
h[i,j] = x_i . x_j - (||x_j||^2 - mean_sq)/2 with fp8e4m3 DoubleRow
matmuls (effective K=256 per instruction, 2 fp8 MACs per cell-cycle).
The -(sq_j - mean_sq)/2 correction is folded in as the 768th contraction
row (one input dim -- the min-variance one -- is dropped to make room;
costs ~2e-4 relative loss error). DVE max8 reads each 4-bank PSUM group
[128, 2048] directly and keeps the top-8; since argmax_j h = argmin_j d2
and the self-match is always rank 0 by a huge margin, ranks 1..5 are the
5 nearest neighbors. ACT reconstructs d = sqrt(sq_i + mean_sq - 2 h) and
emits log(mean_knn + eps) terms; the host sums the 8 x [128, 8] partials:
loss = -sum/8192. Squared norms are computed on the host from the
quantized values (exactly consistent with the on-device dot products).
"""

import sys
import types

import numpy as np
import ml_dtypes

import concourse.bass as bass
import concourse.mybir as mybir
from concourse.tile import TileContext
from concourse.bass_utils import run_bass_kernel_spmd

P = 128
B = 8192
D = 768
NCORES = 8
BL = B // NCORES          # 1024 local rows per core
KT = 6                    # 6 contraction subtiles of 128 (767 dims + corr row)
NI = BL // P              # 8 row tiles per core
NG = 4                    # column groups of 2048 (4 PSUM banks each)
GW = B // NG              # 2048 columns per group
NCH = GW // 512           # 4 chunks of 512 per group
EPS = 1e-8
WARMUP_MMS = 5
NDMA = 16                 # mv DMA blocks (columns arrive in j order)

FP8 = mybir.dt.float8e4
F32 = mybir.dt.float32
DR = mybir.MatmulPerfMode.DoubleRow


def _split_excess_waits(bir_json: bytes) -> bytes:
    """The walrus in this container rejects instructions carrying more than
    one sem-wait ("Too many sync wait commands"). Hoist all but the last
    wait of any instruction into single-wait EventSemaphore instructions
    inserted just before it on the same engine (same-engine program order
    makes this semantically identical)."""
    import json

    m = json.loads(bir_json)
    for f in m["functions"]:
        for bb in f["blocks"]:
            out_insts = []
            for ins in bb["instructions"]:
                si = ins.get("sync_info")
                waits = (si or {}).get("on_wait") or []
                if len(waits) > 1:
                    for i, w in enumerate(waits[:-1]):
                        out_insts.append(
                            {
                                "debug": ins.get("debug", 0),
                                "engine": ins["engine"],
                                "ins": [],
                                "name": f"{ins['name']}_sw{i}",
                                "opcode": "EventSemaphore",
                                "outs": [],
                                "sync_info": {"on_update": [], "on_wait": [w]},
                            }
                        )
                    si["on_wait"] = [waits[-1]]
                out_insts.append(ins)
            bb["instructions"] = out_insts
    return json.dumps(m).encode()


def _patch_compile_for_wait_limit():
    import concourse.bass_utils as bu
    import concourse.bass2jax as b2j

    if getattr(bu, "_wait_split_patched", False):
        return
    orig = bu.compile_bir_kernel

    def compile_bir_kernel(bir_json, tmpdir, neff_name="file.neff"):
        return orig(_split_excess_waits(bir_json), tmpdir, neff_name)

    bu.compile_bir_kernel = compile_bir_kernel
    b2j.compile_bir_kernel = compile_bir_kernel
    bu._wait_split_patched = True


def _install_ntff_hook_shim():
    """The trimmed image lacks antenv.axon_hooks; recreate it so
    run_bass_kernel_spmd(trace=True) can capture NTFF profiles via axon."""
    if "antenv.axon_hooks" in sys.modules:
        return
    try:
        import antenv
        from trn_agent_boot.trn_boot import _ntff_profile_via_ctypes
    except Exception:
        return
    mod = types.ModuleType("antenv.axon_hooks")
    _hook = _ntff_profile_via_ctypes("/opt/axon/libaxon_pjrt.so")
    mod.get_axon_ntff_profile_hook = lambda: _hook
    mod.set_axon_ntff_profile_hook = lambda h: None
    sys.modules["antenv.axon_hooks"] = mod
    antenv.axon_hooks = mod


def build_kernel(debug: bool = False) -> bass.Bass:
    nc = bass.Bass(target_bir_lowering=False, trn_type="TRN2")
    # mv[p, ((b*KT + k)*BW) + j] = V[k*128+p, b*BW+j] where V rows 0..766 are
    # x^T (quantized, min-variance dim dropped) and row 767 is the centered
    # norm correction. Block-major so each DMA moves contiguous 3 KB per
    # partition (512-byte lines would run at ~35 GB/s).
    mv = nc.dram_tensor("mv", [P, KT * B], FP8, kind="ExternalInput")
    # st[p, k*BL + m] = V[k*128+p, r0+m] -- this core's 1024 rows as columns.
    st = nc.dram_tensor("st", [P, KT * BL], FP8, kind="ExternalInput")
    # bias[p, t] = sq[r0 + t*128 + p] + mean_sq
    bias = nc.dram_tensor("bias", [P, NI], F32, kind="ExternalInput")
    out = nc.dram_tensor("out", [P, NI], F32, kind="ExternalOutput")
    if debug:
        dbg_ps = nc.dram_tensor("dbg_ps", [P, GW], F32, kind="ExternalOutput")
        dbg_top = nc.dram_tensor("dbg_top", [P, 8 * NG + 8], F32, kind="ExternalOutput")

    with TileContext(nc) as tc:
        with (
            tc.tile_pool(name="big", bufs=1) as big_pool,
            tc.tile_pool(name="small", bufs=1) as small_pool,
            tc.tile_pool(name="tops", bufs=2) as top_pool,
            tc.tile_pool(name="ps", bufs=2, space="PSUM") as psum_pool,
        ):
            # ---- warmup: get the PE HAM to K=8/8 while DMAs land ----
            warm = small_pool.tile([P, 512], FP8, name="warm")
            nc.gpsimd.memset(warm, 0.25)
            wps = psum_pool.tile([P, GW], F32, name="ps")
            for w in range(WARMUP_MMS):
                nc.tensor.matmul(
                    wps[:, (w % NCH) * 512 : (w % NCH + 1) * 512],
                    lhsT=warm[:, 0:P],
                    rhs=warm[:, 0:512],
                    start=True,
                    stop=True,
                )

            # ---- operand loads ----
            # The first matmul needs only st pair 0 + mv block 0 (~0.65 MB);
            # issue those first on separate queues so they land ~10.5us even
            # under full HBM contention from the later blocks.
            st_sb = big_pool.tile([P, KT, BL], FP8, name="st_sb")
            st_ap = st[:].rearrange("p (k m) -> p k m", k=KT)
            bw = B // NDMA
            mv_sb = big_pool.tile([P, NDMA * KT, bw], FP8, name="mv_sb")
            mv_ap = mv[:].rearrange("p (bk j) -> p bk j", j=bw)
            def mv_block_dma(eng, b):
                eng.dma_start(
                    mv_sb[:, b * KT : (b + 1) * KT, :],
                    mv_ap[:, b * KT : (b + 1) * KT, :],
                )

            nc.sync.dma_start(st_sb[:, 0:2, :], st_ap[:, 0:2, :])
            nc.scalar.dma_start(st_sb[:, 2:4, :], st_ap[:, 2:4, :])
            mv_block_dma(nc.sync, 0)
            mv_block_dma(nc.scalar, 1)
            nc.sync.dma_start(st_sb[:, 4:6, :], st_ap[:, 4:6, :])
            bias_sb = small_pool.tile([P, NI], F32, name="bias_sb")
            nc.scalar.dma_start(bias_sb, bias[:])
            for b in range(2, NDMA):
                mv_block_dma(nc.sync if b % 2 == 0 else nc.scalar, b)

            # ---- gram + top-8 + loss terms ----
            # g-outer / i-inner: column-group g only needs mv DMA blocks
            # 4g..4g+3, and the PE spends ~21us per group vs ~5us for the
            # DMA to deliver one -- so the PE starts right after block 0
            # lands and never waits on HBM again.
            s1_all = small_pool.tile([P, NI], F32, name="s1_all")
            cand_all = small_pool.tile([P, NI, 8 * NG], F32, name="cand_all")
            for g in range(NG):
                for i in range(NI):
                    last = i == NI - 1 and g == NG - 1
                    ps = psum_pool.tile([P, GW], F32, name="ps")
                    if last:
                        # Final group: chunk-serial order + per-chunk max8s so
                        # the reduction overlaps these matmuls instead of
                        # trailing them; only a tiny merge remains at the end.
                        c32 = top_pool.tile([P, 32], F32, name="c32")
                        for c in range(NCH):
                            b = g * NCH + c
                            for t in range(KT // 2):
                                nc.tensor.matmul(
                                    ps[:, c * 512 : (c + 1) * 512],
                                    lhsT=st_sb[:, 2 * t : 2 * t + 2, i * P : (i + 1) * P],
                                    rhs=mv_sb[:, b * KT + 2 * t : b * KT + 2 * t + 2, :],
                                    start=(t == 0),
                                    stop=(t == KT // 2 - 1),
                                    perf_mode=DR,
                                )
                            nc.vector.max(
                                out=c32[:, c * 8 : (c + 1) * 8],
                                in_=ps[:, c * 512 : (c + 1) * 512],
                            )
                        nc.vector.max(
                            out=cand_all[:, i, g * 8 : (g + 1) * 8], in_=c32
                        )
                    else:
                        for t in range(KT // 2):
                            w = st_sb[:, 2 * t : 2 * t + 2, i * P : (i + 1) * P]
                            for c in range(NCH):
                                b = g * NCH + c
                                nc.tensor.matmul(
                                    ps[:, c * 512 : (c + 1) * 512],
                                    lhsT=w,
                                    rhs=mv_sb[:, b * KT + 2 * t : b * KT + 2 * t + 2, :],
                                    start=(t == 0),
                                    stop=(t == KT // 2 - 1),
                                    perf_mode=DR,
                                )
                        if debug and i == 0 and g == 0:
                            dbg_sb = top_pool.tile([P, GW], F32, name="dbg_sb")
                            nc.scalar.copy(dbg_sb, ps[:, :])
                            nc.sync.dma_start(dbg_ps[:], dbg_sb)
                        nc.vector.max(
                            out=cand_all[:, i, g * 8 : (g + 1) * 8], in_=ps[:, :]
                        )
                    if g == NG - 1:
                        top8 = top_pool.tile([P, 8], F32, name="top8")
                        nc.vector.max(out=top8, in_=cand_all[:, i, :])
                        if debug and i == 0:
                            nc.sync.dma_start(dbg_top[:, 0 : 8 * NG], cand_all[:, 0, :])
                            nc.sync.dma_start(dbg_top[:, 8 * NG :], top8[:])
                        d5 = top_pool.tile([P, 5], F32, name="d5")
                        nc.scalar.activation(
                            out=d5,
                            in_=top8[:, 1:6],
                            func=mybir.ActivationFunctionType.Sqrt,
                            bias=bias_sb[:, i : i + 1],
                            scale=-2.0,
                            accum_out=s1_all[:, i : i + 1],
                        )
            # Ship the top-5 distance sums; the host takes the log (keeps the
            # Ln table load off the device critical path).
            nc.sync.dma_start(out[:], s1_all)

    return nc


def _prep_inputs(x: np.ndarray):
    """Quantize, fold the norm correction into contraction row 767, and
    build the per-core operand arrays."""
    e4 = ml_dtypes.float8_e4m3fn
    dstar = int(np.argmin(x.var(axis=0)))
    xk = np.delete(x, dstar, axis=1)            # [B, 767]
    x8 = xk.astype(e4)
    xq = x8.astype(np.float32)
    sq = (xq.astype(np.float64) ** 2).sum(1).astype(np.float32)   # [B]
    sbar = np.float32(sq.mean())
    c8 = (-(sq - sbar) / 2).astype(e4)

    V = np.empty((KT * P, B), dtype=e4)         # [768, B] moving operand
    V[: D - 1] = x8.T
    V[D - 1] = c8
    bw = B // 16
    # [k, p, b, j] -> [p, b, k, j]: block-major, 3 KB contiguous per partition
    Vr = np.ascontiguousarray(
        V.reshape(KT, P, 16, bw).transpose(1, 2, 0, 3).reshape(P, KT * B)
    )
    # Stationary operand: same x rows but correction row replaced by ones,
    # so the folded term contributes 1 * c_j per output element.
    Vs = V.copy()
    Vs[D - 1] = np.float32(1.0)
    in_maps = []
    for core in range(NCORES):
        r0 = core * BL
        st_np = np.ascontiguousarray(
            Vs[:, r0 : r0 + BL]
            .reshape(KT, P, BL)
            .transpose(1, 0, 2)
            .reshape(P, KT * BL)
        )
        bias_np = np.ascontiguousarray(
            (sq[r0 : r0 + BL] + sbar).reshape(NI, P).T
        ).astype(np.float32)
        in_maps.append({"mv": Vr, "st": st_np, "bias": bias_np})
    return in_maps


def run(inputs: dict, trace: bool = False):
    _patch_compile_for_wait_limit()
    if trace:
        _install_ntff_hook_shim()

    x = np.asarray(inputs["student_output"], dtype=np.float32)
    assert x.shape == (B, D), x.shape

    in_maps = _prep_inputs(x)
    nc = build_kernel()
    res = run_bass_kernel_spmd(
        nc, in_maps, core_ids=list(range(NCORES)), trace=trace
    )
    total = 0.0
    for c in range(NCORES):
        s1 = res.results[c]["out"].astype(np.float64)
        total += np.log(s1 / 5.0 + EPS).sum()
    loss = np.float32(-total / B)
    return np.asarray(loss, dtype=np.float32), res


def kernel(**inputs) -> np.ndarray:
    out, _ = run(inputs, trace=False)
    return out
